# revision 49
# baseline (speedup 1.0000x reference)
"""GroupedQueryAttention (B=2, N=2048, D=2048, H=16, HKV=4, HD=128) on 8 trn2 cores.

Sharding: core c handles (batch b = c//4, kv-head g = c%4): 4 q-heads + 1 kv head.
RoPE (with the reference's sin==cos quirk) is folded into Wq/Wk host-side, so
on-device RoPE is an elementwise multiply by a precomputed cos table. The
softmax scale is folded into Wq. All matmuls run in bf16 with fp32 PSUM.

v4 design (collective-free; PE-saturated: ~91% Tensor occupancy, 292us vs
377us for the v2 AllGather design on the same device state):
  - Attention in transpose-free layout: qT,kT [hd,n]; ST = kT.T @ qT [m,n];
    exp on ScalarE; OT += v.T @ ST.
  - Softmax denominators WITHOUT per-tile ones-matmuls: exp tiles are
    accumulated on the DVE (bf16 adds), then ONE ones-matmul per (head,chunk)
    contracts the 128 partitions (errors wash out across partitions).
  - NO collectives: each core out-projects only its LOCAL 512 attention-output
    rows against its [512, 2048] row-slice of Wo, producing a full-width
    PARTIAL output [2048(d), N] in bf16; the host sums the 4 partials per
    batch (the "all-reduce after out_proj" of the sharding hint, done at
    unshard time). Same PE columns as the gathered form (4 jc x 16 dc vs
    16 jc x 4 dc), but no CC-core time, no gather readbacks, no tail stall.
  - n is processed in chunks [256,512,512,448,320]; Q-projections of later
    chunks and partial-out-projection (slab) matmuls of earlier chunks are
    interleaved into each chunk's attention PE stream, dealt per-op-range so
    every chunk's PE work exceeds its ACT (exp) work: exp costs
    ~0.833ns/col + ~217ns/instr, so narrow chunks are ACT-heavy and get
    extra fill; ~20% of the last chunk's fill is held past its nominal
    slots to cover the final head's ACT lag.
  - K/V projections are streamed per-contraction-chunk under the x DMA.
  - DMA discipline: one DMA_DIRECT2D is striped across all 16 hw engines but
    costs ~650ns of issue time on the sync engine, so few large transfers,
    issued in consumption order on the one hw queue whose FIFO data order
    gives just-in-time arrival (wk, x-kd0 first half, wv, rest of x; cos+wq
    slotted in after kd12 right before the Q projections need them; wo
    last). Weights are passed from the host pre-rearranged partition-major
    so every DMA line is >=4KB contiguous (256B lines move ~2x slower).
    Out-projection results stage through 4-dc-wide SBUF tiles -> one DMA
    per group (2-dc for the last chunk so the final flush pipelines out).
Host gathers: out[b] = (sum of the 4 cores' [2048, N] partials).T
"""

import sys
import types

import numpy as np

B, N, D = 2, 2048, 2048
H, HKV, HD = 16, 4, 128
G = H // HKV  # q heads per kv head = 4
N_CORES = 8
ROPE_BASE = 10000.0
JL = G * HD  # 512 local attention-output rows per core

CHUNKS = [(0, 256), (256, 512), (768, 512), (1280, 448), (1728, 320)]


def _install_axon_ntff_hook():
    """This container's antenv lacks axon_hooks; inject it so trace=True works."""
    if "antenv.axon_hooks" in sys.modules:
        return
    try:
        from trn_agent_boot.trn_boot import _ntff_profile_via_ctypes

        hook = _ntff_profile_via_ctypes("/opt/axon/libaxon_pjrt.so")
    except Exception:
        hook = None
    mod = types.ModuleType("antenv.axon_hooks")
    mod.get_axon_ntff_profile_hook = lambda: hook
    mod.set_axon_ntff_profile_hook = lambda h: None
    sys.modules["antenv.axon_hooks"] = mod


def _fold_rope(w: np.ndarray, n_heads: int) -> np.ndarray:
    """Return W' with the (sin==cos) RoPE mixing folded in: x@W' = M(x@W) per head."""
    wf = w.reshape(D, n_heads, HD)
    lo, hi = wf[..., : HD // 2], wf[..., HD // 2 :]
    return np.concatenate([lo - hi, hi + lo], axis=-1).reshape(D, n_heads * HD)


def _cos_table() -> np.ndarray:
    inv_freq = 1.0 / (ROPE_BASE ** (np.arange(0, HD, 2, dtype=np.float64) / HD))
    freqs = np.arange(N, dtype=np.float64)[:, None] * inv_freq[None, :]  # [N, 64]
    emb = np.concatenate([freqs, freqs], axis=-1)  # [N, 128]
    return np.cos(emb).T.astype(np.float32).copy()  # [128, N]


_NC_CACHE: dict = {}


def _build_nc():
    if "nc" in _NC_CACHE:
        return _NC_CACHE["nc"]

    import concourse.bacc as bacc
    import concourse.mybir as mybir
    import concourse.tile as tile
    from concourse.bass import ts
    from concourse.masks import make_identity

    f32 = mybir.dt.float32
    bf16 = mybir.dt.bfloat16
    AFT = mybir.ActivationFunctionType
    KD = D // 128  # 16 contraction chunks
    NT = N // 128  # 16 m tiles of 128
    NC512 = N // 512
    DC = D // 128  # 16 d-tiles of the full-width partial out-projection

    nc = bacc.Bacc(target_bir_lowering=False, debug=False, num_devices=N_CORES)

    # weights arrive pre-rearranged partition-major from the host so every
    # DMA line is >=4KB contiguous (256B lines measured ~2x slower)
    xt = nc.dram_tensor("xt", [D, N], bf16, kind="ExternalInput")  # x[b].T
    wq = nc.dram_tensor("wq", [128, KD, JL], bf16, kind="ExternalInput")
    wk = nc.dram_tensor("wk", [128, KD, HD], bf16, kind="ExternalInput")
    wv = nc.dram_tensor("wv", [128, KD, HD], bf16, kind="ExternalInput")
    wo = nc.dram_tensor("wo", [128, G, D], bf16, kind="ExternalInput")
    cost = nc.dram_tensor("cost", [HD, N], bf16, kind="ExternalInput")
    # transposed PARTIAL output: outT[d, n] bf16; host sums partials + transposes
    out = nc.dram_tensor("out", [D, N], bf16, kind="ExternalOutput")

    xt_v = xt.rearrange("(ko p) n -> p ko n", p=128)
    wq_v = wq
    wk_v = wk
    wv_v = wv
    wo_v = wo
    out_v = out.rearrange("(dc p) n -> p dc n", p=128)

    with tile.TileContext(nc) as tc:
        with (
            tc.tile_pool(name="big", bufs=1) as big_pool,
            tc.tile_pool(name="wpool", bufs=1) as w_pool,
            tc.tile_pool(name="work", bufs=1) as work_pool,
            tc.tile_pool(name="st", bufs=8) as st_pool,
            tc.tile_pool(name="acc", bufs=2) as acc_pool,
            tc.tile_pool(name="otn", bufs=3) as otn_pool,
            tc.tile_pool(name="recip", bufs=1) as recip_pool,
            tc.tile_pool(name="osb", bufs=4) as osb_pool,
            tc.tile_pool(name="psS", bufs=2, space="PSUM") as psS,
            tc.tile_pool(name="psOT", bufs=2, space="PSUM") as psOT,
            tc.tile_pool(name="psP", bufs=2, space="PSUM") as psP,
            tc.tile_pool(name="psSum", bufs=1, space="PSUM") as psSum,
            tc.tile_pool(name="psQ", bufs=1, space="PSUM") as psQ,
        ):
            # ---- persistent SBUF tensors ----
            x_sb = big_pool.tile([128, KD, N], bf16, tag="big")
            wq_sb = w_pool.tile([128, KD, JL], bf16, tag="wq")
            wk_sb = w_pool.tile([128, KD, HD], bf16, tag="wk")
            wv_sb = w_pool.tile([128, KD, HD], bf16, tag="wv")
            wo_sb = w_pool.tile([128, G, D], bf16, tag="wo")
            cos_sb = w_pool.tile([128, N], bf16, tag="cos")
            qT_sb = work_pool.tile([128, G, N], bf16, tag="qT")
            kT_sb = work_pool.tile([128, N], bf16, tag="kT")
            vT_sb = work_pool.tile([128, N], bf16, tag="vT")
            v_sb = work_pool.tile([128, N], bf16, tag="v")  # [m-part, mt*128+hd]
            ones_sb = work_pool.tile([128, 128], bf16, tag="ones")
            ident_sb = work_pool.tile([128, 128], bf16, tag="ident")

            nc.gpsimd.memset(ones_sb[:], 1.0)
            make_identity(nc, ident_sb[:])

            # ---- input DMAs (consumption order) ----
            # one DMA_DIRECT2D is already striped across all 16 hw engines,
            # but each costs ~650ns of ISSUE time on the issuing engine —
            # so: few, large DMAs, all on the hw sync queue, whose FIFO data
            # order we exploit for just-in-time arrival: startup is
            # DMA-bandwidth-bound (~12MB at ~320 GB/s), so cos+wq slot into
            # the x stream right before the Q0 projection needs them, and wo
            # (first needed at chunk-1's out-projection, ~90us in) goes last.
            # kd0 in halves between wk and wv: the first K-proj matmuls need
            # only wk + kd0's first half, so they start ~2us sooner
            nc.sync.dma_start(wk_sb[:], wk_v[:])
            nc.sync.dma_start(x_sb[:, 0, ts(0, 1024)], xt_v[:, 0, ts(0, 1024)])
            nc.sync.dma_start(wv_sb[:], wv_v[:])
            nc.sync.dma_start(x_sb[:, 0, ts(1, 1024)], xt_v[:, 0, ts(1, 1024)])
            for kd in range(1, 13):
                nc.sync.dma_start(x_sb[:, kd, :], xt_v[:, kd, :])
            nc.sync.dma_start(cos_sb[:], cost[:])
            for s in range(2):
                nc.sync.dma_start(wq_sb[:, ts(s, 8), :], wq_v[:, ts(s, 8), :])
            for kd in range(13, KD):
                nc.sync.dma_start(x_sb[:, kd, :], xt_v[:, kd, :])
            for s in range(2):
                nc.sync.dma_start(wo_sb[:, ts(s, 2), :], wo_v[:, ts(s, 2), :])

            # PE clock warm-up: the PE runs its first ~3us of any busy
            # stretch at reduced clock; these no-dependency matmuls (NOT
            # transposes — interleaved transposes measured a global ~20%
            # matmul slowdown) spin the engine during the otherwise-idle
            # wait for wk + the first x chunk, so the K/V chain starts at
            # full clock. MUST be the first psP allocation — a later one
            # would rotate the pool against psK tiles whose release is
            # queued behind it.
            # ~44 ops bridge the full wait until kd0 data (~12us) — ending
            # early lets the PE idle and resets the clock ramp
            warm = psP.tile([128, 512], f32, tag="p", name="warm")
            for i in range(44):
                nc.tensor.matmul(
                    warm[:, ts(i % 4, 128)],
                    lhsT=ident_sb[:],
                    rhs=ones_sb[:],
                    start=True,
                    stop=True,
                )

            # ---- K+V projections streamed per kd chunk under the x DMA ----
            # (borrows all 8 PSUM banks; phase-exclusive with attention)
            psK = [
                psOT.tile([128, 512], f32, tag="ot", name="psK0"),
                psOT.tile([128, 512], f32, tag="ot", name="psK1"),
                psP.tile([128, 512], f32, tag="p", name="psK2"),
                psP.tile([128, 512], f32, tag="p", name="psK3"),
            ]
            # psQ first: the chunk-0 Q projection (emitted right after the
            # K/V projections) blocks on psQ, which the FIRST vT copy frees
            psV = [
                psQ.tile([128, 512], f32, tag="q", name="psV0"),
                psS.tile([128, 512], f32, tag="mm", name="psV1"),
                psS.tile([128, 512], f32, tag="mm", name="psV2"),
                psSum.tile([128, 512], f32, tag="sums", name="psV3"),
            ]
            for kd in range(KD):
                for ncx in range(NC512):
                    nc.tensor.matmul(
                        psK[ncx],
                        lhsT=wk_sb[:, kd, :],
                        rhs=x_sb[:, kd, ts(ncx, 512)],
                        start=(kd == 0),
                        stop=(kd == KD - 1),
                    )
                for ncx in range(NC512):
                    nc.tensor.matmul(
                        psV[ncx],
                        lhsT=wv_sb[:, kd, :],
                        rhs=x_sb[:, kd, ts(ncx, 512)],
                        start=(kd == 0),
                        stop=(kd == KD - 1),
                    )
            for ncx in range(NC512):
                nc.vector.tensor_copy(vT_sb[:, ts(ncx, 512)], psV[ncx])
            for ncx in range(NC512):
                nc.vector.tensor_mul(
                    kT_sb[:, ts(ncx, 512)], psK[ncx], cos_sb[:, ts(ncx, 512)]
                )

            def q_ops(ci):
                """Q-projection of chunk ci as a list of single-op closures."""
                o, w = CHUNKS[ci]
                ops = []
                state = {}

                def mk_mm(h, kd):
                    def op():
                        if kd == 0:
                            state[h] = psQ.tile(
                                [128, 512], f32, tag="q", name=f"psq{ci}_{h}"
                            )
                        nc.tensor.matmul(
                            state[h][:, :w],
                            lhsT=wq_sb[:, kd, ts(h, 128)],
                            rhs=x_sb[:, kd, o : o + w],
                            start=(kd == 0),
                            stop=(kd == KD - 1),
                        )
                        if kd == KD - 1:
                            nc.vector.tensor_mul(
                                qT_sb[:, h, o : o + w],
                                state.pop(h)[:, :w],
                                cos_sb[:, o : o + w],
                            )

                    return op

                for h in range(G):
                    for kd in range(KD):
                        ops.append(mk_mm(h, kd))
                return ops

            q_cache = {}

            def q_range(cj, lo, hi):
                if cj not in q_cache:
                    q_cache[cj] = q_ops(cj)
                return q_cache[cj][lo:hi]

            # ---- local partial out-projection (slab) per chunk ----
            otn_tiles = {}

            def slab_ops(ci):
                """Partial out-proj of chunk ci (local 512 rows) as closures.

                Contract jc over the 4 local heads; 16 dc tiles of 128 output
                d-rows each; result DMA'd to the [D, N] partial in HBM.
                """
                o, w = CHUNKS[ci]
                # 4-dc-wide staging + one DMA per group amortizes the sync
                # engine's ~650ns per-issue cost; the LAST chunk uses 2-dc
                # groups so the final HBM flush pipelines with the last
                # matmuls instead of draining after them
                gsz = 2 if ci == len(CHUNKS) - 1 else 4
                ops = []
                state = {}
                osb = {}
                otn_ch = otn_tiles.pop(ci)

                def mk_mm(dc, jc):
                    def op():
                        if jc == 0:
                            state[dc] = psP.tile(
                                [128, 512], f32, tag="p", name=f"psp{ci}_{dc}"
                            )
                        nc.tensor.matmul(
                            state[dc][:, :w],
                            lhsT=wo_sb[:, jc, ts(dc, 128)],
                            rhs=otn_ch[:, jc, :w],
                            start=(jc == 0),
                            stop=(jc == G - 1),
                        )
                        if jc == G - 1:
                            if dc % gsz == 0:
                                osb[dc // gsz] = osb_pool.tile(
                                    [128, gsz, 512],
                                    bf16,
                                    tag="osb",
                                    name=f"osb{ci}_{dc // gsz}",
                                )
                            o_sb = osb[dc // gsz]
                            nc.vector.tensor_copy(
                                o_sb[:, dc % gsz, :w], state.pop(dc)[:, :w]
                            )
                            if dc % gsz == gsz - 1:
                                nc.sync.dma_start(
                                    out_v[:, ts(dc // gsz, gsz), o : o + w],
                                    osb.pop(dc // gsz)[:, :, :w],
                                )

                    return op

                for dc in range(DC):
                    for jc in range(G):
                        ops.append(mk_mm(dc, jc))
                return ops

            # vT [hd, m] -> v [m-part, hd] via PE transpose (after Q0 in the
            # PE stream: Q0 needs only psQ + wq; the transposes' psP banks
            # free up while Q0 runs, and v_sb is not needed until the first
            # attnV matmul)
            def transpose_ops(q4):
                box = {}

                def mk(j):
                    def op():
                        if j == 0:
                            box["t"] = psP.tile(
                                [128, 512], bf16, tag="p", name=f"pst{q4}"
                            )
                        nc.tensor.transpose(
                            box["t"][:, ts(j, 128)],
                            vT_sb[:, ts(q4 * 4 + j, 128)],
                            ident_sb[:],
                        )
                        if j == 3:
                            nc.vector.tensor_copy(v_sb[:, ts(q4, 512)], box.pop("t"))

                    return op

                return [mk(j) for j in range(4)]

            # only head 0 of the chunk-0 Q projection runs serially before
            # attention — heads 1-3 ride chunk-0's fill stream (head h's qT
            # is consumed only at slot 16h, so paced fill stays ahead).
            # ALL transposes stay serial: interleaving PE transposes into
            # the attention-era stream measured a global ~20% matmul
            # slowdown (four reproductions; mechanism unclear — suspected
            # transpose-mode/pipeline state churn)
            for op in q_range(0, 0, KD):
                op()
            for q4 in range(NT // 4):
                for op in transpose_ops(q4):
                    op()

            # ---- attention chunks ----
            LEAD = 4
            # Q-proj and slab fills are dealt per-op-range so every chunk's
            # PE stream stays RICHER than its ACT (exp) stream — exp tiles
            # carry a ~220ns per-instruction overhead, so narrow chunks are
            # ACT-heavy and need extra PE fill to avoid starving the PE
            # (per-chunk PE-vs-ACT surplus at 0.417ns/col: +3.3, +3.1,
            # +6.6, +3.1, +8.3 us)
            QFILL_AT = {
                0: [(0, 16, 64), (1, 0, 64), (2, 0, 16)],
                1: [(2, 16, 64)],
                2: [(3, 0, 64)],
                3: [(4, 0, 64)],
            }
            SLAB_AT = {
                1: [(0, 0, 64)],
                2: [(1, 0, 40)],
                3: [(1, 40, 64), (2, 0, 16)],
                4: [(2, 16, 64), (3, 0, 64)],
            }
            slab_cache = {}

            def slab_range(cj, lo, hi):
                if cj not in slab_cache:
                    slab_cache[cj] = slab_ops(cj)
                return slab_cache[cj][lo:hi]

            for ci, (o, w) in enumerate(CHUNKS):
                # fills: Q-proj of later chunks paced over all slots; slab of
                # earlier chunks (their otn is complete when this chunk
                # starts; no collective to wait for)
                qfill = [
                    op
                    for (cj, lo, hi) in QFILL_AT.get(ci, [])
                    for op in q_range(cj, lo, hi)
                ]
                sfill = [
                    op
                    for (cj, lo, hi) in SLAB_AT.get(ci, [])
                    for op in slab_range(cj, lo, hi)
                ]
                nslots = G * NT
                # last chunk: hold ~20% of slab fill past the nominal slots —
                # its final head's exps gate the PE, and leftover fill
                # (flushed by the while-loops below) covers that ACT lag
                sslots = nslots + (16 if ci == len(CHUNKS) - 1 else 0)
                otn_ch = otn_pool.tile([128, G, 512], bf16, tag="otn", name=f"otn{ci}")
                otn_tiles[ci] = otn_ch
                qi = si = 0
                slot = 0
                for h in range(G):
                    ot_ps = psOT.tile([128, 512], f32, tag="ot")
                    acc = acc_pool.tile([128, 512], bf16, tag="acc")
                    st_prev = None
                    for mt in range(NT):
                        s_ps = psS.tile([128, 512], f32, tag="mm")
                        nc.tensor.matmul(
                            s_ps[:, :w],
                            lhsT=kT_sb[:, ts(mt, 128)],
                            rhs=qT_sb[:, h, o : o + w],
                            start=True,
                            stop=True,
                        )
                        st_sb = st_pool.tile([128, 512], bf16, tag="st")
                        nc.scalar.activation(st_sb[:, :w], s_ps[:, :w], AFT.Exp)
                        nc.tensor.matmul(
                            ot_ps[:, :w],
                            lhsT=v_sb[:, ts(mt, 128)],
                            rhs=st_sb[:, :w],
                            start=(mt == 0),
                            stop=(mt == NT - 1),
                        )
                        if mt == 1:
                            nc.vector.tensor_add(
                                acc[:, :w], st_prev[:, :w], st_sb[:, :w]
                            )
                        elif mt >= 2:
                            nc.vector.tensor_add(acc[:, :w], acc[:, :w], st_sb[:, :w])
                        st_prev = st_sb
                        # interleave Q-proj(ci+1) / out-proj(ci-1) into the
                        # ACT-bound attention stream
                        slot += 1
                        qt = (len(qfill) * slot) // nslots
                        while qi < qt:
                            qfill[qi]()
                            qi += 1
                        st_ = (len(sfill) * max(0, slot - LEAD)) // (sslots - LEAD)
                        while si < st_:
                            sfill[si]()
                            si += 1
                    sums_ps = psSum.tile([128, 512], f32, tag="sums")
                    nc.tensor.matmul(
                        sums_ps[:, :w],
                        lhsT=ones_sb[:],
                        rhs=acc[:, :w],
                        start=True,
                        stop=True,
                    )
                    recip_sb = recip_pool.tile([128, 512], f32, tag="recip")
                    nc.vector.reciprocal_approx_fast(recip_sb[:, :w], sums_ps[:, :w])
                    nc.vector.tensor_mul(
                        otn_ch[:, h, :w], ot_ps[:, :w], recip_sb[:, :w]
                    )
                while qi < len(qfill):
                    qfill[qi]()
                    qi += 1
                while si < len(sfill):
                    sfill[si]()
                    si += 1

            # tail: partial out-proj of the last chunk
            for op in slab_range(len(CHUNKS) - 1, 0, DC * G):
                op()

    nc.compile()
    _NC_CACHE["nc"] = nc
    return nc


def kernel(x, Wq, Wk, Wv, Wo):
    _install_axon_ntff_hook()
    import ml_dtypes

    import concourse.bass_utils as bass_utils

    bass_utils.upload_artifacts = lambda tmpdir: str(tmpdir)
    from concourse.bass_utils import run_bass_kernel_spmd

    x = np.asarray(x, dtype=np.float32)
    Wq = np.asarray(Wq, dtype=np.float32)
    Wk = np.asarray(Wk, dtype=np.float32)
    Wv = np.asarray(Wv, dtype=np.float32)
    Wo = np.asarray(Wo, dtype=np.float32)

    bf = ml_dtypes.bfloat16
    scale = np.float32(HD**-0.5)
    wq_f = (_fold_rope(Wq, H) * scale).astype(bf)  # [D, 2048]
    wk_f = _fold_rope(Wk, HKV).astype(bf)  # [D, 512]
    wv_f = Wv.astype(bf)  # [D, 512]
    wo_f = Wo.astype(bf)  # [2048, D]
    cos_t = _cos_table().astype(bf)  # [128, N] bf16

    xt = [np.ascontiguousarray(x[b].T).astype(bf) for b in range(B)]

    def pmajor(a, cols):
        """[D, cols] -> [128, D//128, cols] partition-major (contiguous lines)."""
        return np.ascontiguousarray(a.reshape(D // 128, 128, cols).transpose(1, 0, 2))

    in_maps = []
    for c in range(N_CORES):
        b, g = divmod(c, HKV)
        in_maps.append(
            {
                "xt": xt[b],
                "wq": pmajor(wq_f[:, g * JL : (g + 1) * JL], JL),
                "wk": pmajor(wk_f[:, g * HD : (g + 1) * HD], HD),
                "wv": pmajor(wv_f[:, g * HD : (g + 1) * HD], HD),
                "wo": np.ascontiguousarray(
                    wo_f[g * JL : (g + 1) * JL, :]
                    .reshape(G, 128, D)
                    .transpose(1, 0, 2)
                ),
                "cost": cos_t,
            }
        )

    nc = _build_nc()
    res = run_bass_kernel_spmd(nc, in_maps, list(range(N_CORES)))

    out = np.empty((B, N, D), dtype=np.float32)
    for b in range(B):
        acc = res.results[b * HKV]["out"].astype(np.float32)
        for g in range(1, HKV):
            acc += res.results[b * HKV + g]["out"].astype(np.float32)
        out[b] = acc.T
    return out
